# revision 1
# baseline (speedup 1.0000x reference)
"""DCNv2 (deformable conv + BN + ReLU) Trainium2 Bass kernel, 8-core SPMD.

Sharding: core c owns sample b=c//4, output rows [24*(c%4), 24*(c%4)+24).
Pipeline per core:
  1. offset conv (PE, bf16)  -> om[27, pos]
  2. coefficients on DVE/ACT -> bilinear weights a[pos, k, j], gather idx
  3. dma_gather of 2KB 4-corner rows from HBM table (bf16)
  4. scale+transpose+corner-sum fused on PE: S[c9,pos] += G_kj^T @ diag(a_kj)
  5. main GEMM (PE, bf16):  out[o,pos] = sum_ch W'[ch].T @ S[ch]
  6. BN stats AllReduce (8 cores), scale/shift/ReLU on ACT.
"""

import numpy as np
import ml_dtypes

BF16 = ml_dtypes.bfloat16
B, CI, CO, H, W = 2, 256, 256, 96, 96
NCORES = 8
RB = 24                      # output rows per core
NPOS = RB * W                # 2304 positions per core
PADG = 8                     # gather-table pad on each side
GRID = H + 2 * PADG          # 112
NROWS = GRID * GRID          # 12544 table rows
NTOT = float(B * H * W)      # BN count
EPS = 1e-5

KY9 = np.repeat(np.arange(3), 3).astype(np.float32)
KX9 = np.tile(np.arange(3), 3).astype(np.float32)

_CACHE = {}


def _build_program():
    import concourse.bass as bass
    from concourse import bacc, tile, mybir

    ds = bass.ds
    f32 = mybir.dt.float32
    bf16 = mybir.dt.bfloat16
    i16 = mybir.dt.int16
    Alu = mybir.AluOpType
    Act = mybir.ActivationFunctionType

    nc = bacc.Bacc("TRN2", target_bir_lowering=False, debug=False,
                   num_devices=NCORES)

    # ---- external inputs (per-core values supplied in in_maps) ----
    tab_d = nc.dram_tensor("tab", [NROWS, 1024], bf16, kind="ExternalInput")
    slab_d = nc.dram_tensor("slab", [128, 2, RB + 2, W + 2], bf16,
                            kind="ExternalInput")
    woff_d = nc.dram_tensor("woff", [128, 2, 9, 27], bf16,
                            kind="ExternalInput")
    pypx_d = nc.dram_tensor("pypx", [96, 24, 27], f32, kind="ExternalInput")
    wdcn_d = nc.dram_tensor("wdcn", [128, 18, 2, 128], bf16,
                            kind="ExternalInput")
    ident_d = nc.dram_tensor("ident", [128, 128], bf16, kind="ExternalInput")
    gb_d = nc.dram_tensor("gb", [128, 2, 3], f32, kind="ExternalInput")
    out_d = nc.dram_tensor("out", [2, 128, NPOS], f32, kind="ExternalOutput")

    with tile.TileContext(nc) as tc:
        with (
            tc.tile_pool(name="cst", bufs=1) as cst,
            tc.tile_pool(name="sb", bufs=1) as sb,
            tc.tile_pool(name="gpool", bufs=3) as gpool,
            tc.tile_pool(name="spool", bufs=2) as spool,
            tc.tile_pool(name="dpool", bufs=2) as dpool,
            tc.tile_pool(name="ps_s", bufs=2, space="PSUM") as ps_s,
            tc.tile_pool(name="ps_o", bufs=1, space="PSUM") as ps_o,
            tc.tile_pool(name="dram", bufs=1, space="DRAM") as dram,
        ):
            # ---------- load persistent tiles ----------
            slab = cst.tile([128, 2, RB + 2, W + 2], bf16)
            nc.sync.dma_start(slab[:], slab_d[:])
            woff = cst.tile([128, 2, 9, 27], bf16)
            nc.sync.dma_start(woff[:], woff_d[:])
            pypx = cst.tile([96, 24, 27], f32)
            nc.sync.dma_start(pypx[:], pypx_d[:])
            wdcn = cst.tile([128, 18, 2, 128], bf16)
            nc.sync.dma_start(wdcn[:], wdcn_d[:])
            ident = cst.tile([128, 128], bf16)
            nc.sync.dma_start(ident[:], ident_d[:])
            gb = cst.tile([128, 2, 3], f32)
            nc.sync.dma_start(gb[:], gb_d[:])

            # ---------- phase 1: offset conv, c-part [27, pos] ----------
            om_c = sb.tile([27, 6, 384], f32)
            with tc.tile_pool(name="ps_om", bufs=2, space="PSUM") as ps_om:
                for T in range(6):
                    pom = ps_om.tile([27, 384], f32)
                    first = True
                    for ct in range(2):
                        for k in range(9):
                            ky, kx = int(KY9[k]), int(KX9[k])
                            rhs = slab[:, ct, T * 4 + ky:T * 4 + ky + 4,
                                       kx:kx + 96]
                            nc.tensor.matmul(pom[:], woff[:, ct, k, :], rhs,
                                             start=first,
                                             stop=(ct == 1 and k == 8))
                            first = False
                    nc.scalar.copy(om_c[:, T, :], pom[:])

            # ---------- phase 2: transpose om to pos-part via DRAM ----------
            om_sc = dram.tile([NPOS, 27], f32)
            # src [27p, 24t, 96w] -> scratch[(t*96+w), c]
            src = om_c[:].rearrange("p a (tl w) -> p (a tl) w", w=96)
            nc.sync.dma_start(om_sc[:].rearrange("(t w) c -> c t w", t=24), src)
            om_pos = sb.tile([96, 24, 27], f32)
            nc.sync.dma_start(
                om_pos[:], om_sc[:].rearrange("(t w) c -> w t c", t=24))

            # ---------- phase 3: coefficients ----------
            opp = sb.tile([96, 24, 27], f32)
            nc.vector.tensor_tensor(opp[:], om_pos[:], pypx[:], Alu.add)
            msk = sb.tile([96, 24, 9], f32)
            nc.scalar.activation(msk[:], opp[:, :, 18:27], Act.Sigmoid)
            pys = sb.tile([96, 24, 9], f32, tag="pys")
            pxs = sb.tile([96, 24, 9], f32, tag="pxs")
            nc.vector.tensor_scalar_add(pys[:], opp[:, :, 0:9], 16.0)
            nc.vector.tensor_scalar_add(pxs[:], opp[:, :, 9:18], 16.0)
            # floor via round(x - 0.5): (x + (2^23 - 0.5)) - 2^23.
            # Exact-integer x floors one low; harmless (bilinear continuity).
            MAGIC = 8388608.0
            fy = sb.tile([96, 24, 9], f32, tag="fy")
            fx = sb.tile([96, 24, 9], f32, tag="fx")
            iyp = sb.tile([96, 24, 9], f32, tag="iyp")
            ixp = sb.tile([96, 24, 9], f32, tag="ixp")
            nc.vector.tensor_scalar(iyp[:], pys[:], MAGIC - 0.5, -MAGIC,
                                    Alu.add, Alu.add)
            nc.vector.tensor_scalar(ixp[:], pxs[:], MAGIC - 0.5, -MAGIC,
                                    Alu.add, Alu.add)
            nc.vector.tensor_tensor(fy[:], pys[:], iyp[:], Alu.subtract)
            nc.vector.tensor_tensor(fx[:], pxs[:], ixp[:], Alu.subtract)
            # clamp to grid [-8..103] -> iyp in [8, 118]
            nc.vector.tensor_scalar(iyp[:], iyp[:], 8.0, 118.0, Alu.max,
                                    Alu.min)
            nc.vector.tensor_scalar(ixp[:], ixp[:], 8.0, 118.0, Alu.max,
                                    Alu.min)
            idxf = sb.tile([96, 24, 9], f32, tag="idxf")
            nc.vector.tensor_scalar(idxf[:], iyp[:], float(GRID), -904.0,
                                    Alu.mult, Alu.add)
            nc.vector.tensor_tensor(idxf[:], idxf[:], ixp[:], Alu.add)
            idx16 = sb.tile([96, 24, 9], i16)
            nc.vector.tensor_copy(idx16[:], idxf[:])
            wy0 = sb.tile([96, 24, 9], f32, tag="wy0")
            wx0 = sb.tile([96, 24, 9], f32, tag="wx0")
            nc.vector.tensor_scalar(wy0[:], fy[:], -1.0, 1.0, Alu.mult,
                                    Alu.add)
            nc.vector.tensor_scalar(wx0[:], fx[:], -1.0, 1.0, Alu.mult,
                                    Alu.add)
            a96 = sb.tile([96, 24, 4, 9], f32)
            for j, (wy, wx) in enumerate([(wy0, wx0), (wy0, fx),
                                          (fy, wx0), (fy, fx)]):
                nc.vector.tensor_tensor(a96[:, :, j, :], wy[:], wx[:],
                                        Alu.mult)
                nc.vector.tensor_tensor(a96[:, :, j, :], a96[:, :, j, :],
                                        msk[:], Alu.mult)

            # ---------- phase 4: repack idx + a via DRAM ----------
            idx_sc = dram.tile([20736], i16)
            # stream pos: T*3456 + kc*1152 + kk*384 + tl*96 + p
            for T in range(6):
                for k in range(9):
                    src = idx16[:, T * 4:T * 4 + 4, k]
                    dst = idx_sc[ds(T * 3456 + k * 384, 384)].rearrange(
                        "(tl p) -> p tl", tl=4)
                    nc.sync.dma_start(dst, src)
            idxw = sb.tile([128, 6, 216], i16)
            nc.vector.memset(idxw[:], 0)
            nc.sync.dma_start(
                idxw[0:16, :, :].rearrange("r T s -> r (T s)"),
                idx_sc[:].rearrange("(s r) -> r s", r=16))

            a_sc = dram.tile([NPOS, 36], f32)
            nc.sync.dma_start(
                a_sc[:].rearrange("(t p) j -> p t j", t=24),
                a96[:].rearrange("p t j4 k -> p t (j4 k)"))
            a_sb = sb.tile([128, 18, 36], f32)
            nc.sync.dma_start(a_sb[:],
                              a_sc[:].rearrange("(q p) j -> p q j", q=18))

            # ---------- phases 5-7: gather, diag-scale-transpose, GEMM ----
            out_sb = sb.tile([128, 2, NPOS], f32)
            for T in range(6):
                gt = []
                for kc in range(3):
                    g = gpool.tile([128, 9, 1024], bf16, tag="g")
                    nc.gpsimd.dma_gather(
                        g[:], tab_d[:], idxw[:, T, kc * 72:(kc + 1) * 72],
                        num_idxs=1152, num_idxs_reg=1152, elem_size=1024)
                    gt.append(g)
                s_sb = spool.tile([128, 18, 384], bf16, tag="s")
                for q in range(3):
                    qg = T * 3 + q
                    dg = dpool.tile([128, 36, 128], bf16, tag="diag")
                    for kj in range(36):
                        nc.vector.tensor_scalar_mul(
                            dg[:, kj, :], ident[:],
                            a_sb[:, qg, kj:kj + 1])
                    for third in range(3):
                        pss = ps_s.tile([128, 6, 128], f32, tag="pss")
                        for chl in range(6):
                            ch = third * 6 + chl
                            k, cfh = ch // 2, ch % 2
                            g = gt[k // 3]
                            slot = (k % 3) * 3 + q
                            for j in range(4):
                                lhsT = g[:, slot, j * 256 + cfh * 128:
                                         j * 256 + cfh * 128 + 128]
                                nc.tensor.matmul(pss[:, chl, :], lhsT,
                                                 dg[:, (j * 9 + k), :],
                                                 start=(j == 0),
                                                 stop=(j == 3))
                        nc.scalar.copy(
                            s_sb[:, third * 6:third * 6 + 6,
                                 q * 128:(q + 1) * 128], pss[:])
                for o2 in range(2):
                    po = ps_o.tile([128, 384], f32, tag="po")
                    for ch in range(18):
                        nc.tensor.matmul(po[:], wdcn[:, ch, o2, :],
                                         s_sb[:, ch, :], start=(ch == 0),
                                         stop=(ch == 17))
                    nc.vector.tensor_scalar_add(
                        out_sb[:, o2, T * 384:(T + 1) * 384], po[:],
                        gb[:, o2, 2:3])

            # ---------- phase 8: BN stats + allreduce + finish ----------
            part = sb.tile([128, 4], f32)
            scrap = sb.tile([128, NPOS], bf16)
            for o2 in range(2):
                nc.vector.tensor_reduce(part[:, 2 * o2:2 * o2 + 1],
                                        out_sb[:, o2, :],
                                        mybir.AxisListType.X, Alu.add)
                nc.scalar.activation(scrap[:], out_sb[:, o2, :], Act.Square,
                                     accum_out=part[:, 2 * o2 + 1:2 * o2 + 2])
            bin_d = dram.tile([128, 4], f32)
            bout_d = dram.tile([128, 4], f32, addr_space="Shared")
            import os as _os
            nc.gpsimd.dma_start(bin_d[:], part[:])
            if _os.environ.get("NOCC", "0") == "1":
                nc.gpsimd.dma_start(bout_d[:], bin_d[:])
            else:
                nc.gpsimd.collective_compute(
                    "AllReduce", mybir.AluOpType.add,
                    replica_groups=[list(range(NCORES))],
                    ins=[bin_d[:].opt()], outs=[bout_d[:].opt()])
            stats = sb.tile([128, 4], f32)
            nc.sync.dma_start(stats[:], bout_d[:])
            tmp = sb.tile([128, 8], f32)
            outf = sb.tile([128, NPOS], f32)
            for o2 in range(2):
                mean = tmp[:, 4 * o2 + 0:4 * o2 + 1]
                var = tmp[:, 4 * o2 + 1:4 * o2 + 2]
                s_ = tmp[:, 4 * o2 + 2:4 * o2 + 3]
                t_ = tmp[:, 4 * o2 + 3:4 * o2 + 4]
                nc.vector.tensor_scalar_mul(mean, stats[:, 2 * o2:2 * o2 + 1],
                                            1.0 / NTOT)
                nc.vector.tensor_scalar_mul(var,
                                            stats[:, 2 * o2 + 1:2 * o2 + 2],
                                            1.0 / NTOT)
                nc.vector.tensor_tensor(s_, mean, mean, Alu.mult)
                nc.vector.tensor_tensor(var, var, s_, Alu.subtract)
                nc.vector.tensor_scalar_add(var, var, EPS)
                nc.scalar.sqrt(s_, var)
                nc.vector.reciprocal(s_, s_)
                nc.vector.tensor_tensor(s_, s_, gb[:, o2, 0:1], Alu.mult)
                nc.vector.tensor_tensor(t_, mean, s_, Alu.mult)
                nc.vector.tensor_scalar_mul(t_, t_, -1.0)
                nc.vector.tensor_tensor(t_, t_, gb[:, o2, 1:2], Alu.add)
                nc.scalar.activation(outf[:], out_sb[:, o2, :], Act.Relu,
                                     bias=t_, scale=s_)
                nc.sync.dma_start(out_d[o2], outf[:])

    nc.compile()
    return nc


def _prep_inputs(x, w_off, b_off, w_dcn, b_dcn, gamma, beta):
    """Build the 8 per-core input maps (host-side sharding/layout only)."""
    x = np.asarray(x, np.float32)
    w_off = np.asarray(w_off, np.float32)
    b_off = np.asarray(b_off, np.float32)
    w_dcn = np.asarray(w_dcn, np.float32)
    b_dcn = np.asarray(b_dcn, np.float32)
    gamma = np.asarray(gamma, np.float32)
    beta = np.asarray(beta, np.float32)

    # 4-corner gather tables per sample
    P = PADG
    xp = np.zeros((B, CI, GRID + 1, GRID + 1), np.float32)
    xp[:, :, P:P + H, P:P + W] = x
    xp = xp.astype(BF16)
    tabs = []
    for b in range(B):
        t = np.empty((GRID, GRID, 4, CI), BF16)
        for j, (dy2, dx2) in enumerate([(0, 0), (0, 1), (1, 0), (1, 1)]):
            t[:, :, j, :] = np.moveaxis(
                xp[b, :, dy2:dy2 + GRID, dx2:dx2 + GRID], 0, -1)
        tabs.append(np.ascontiguousarray(t.reshape(NROWS, 1024)))

    # conv slab (1-pixel zero pad) per sample, bf16, [128, ct, 26, 98]
    xs = np.zeros((B, CI, H + 2, W + 2), np.float32)
    xs[:, :, 1:H + 1, 1:W + 1] = x
    xs = xs.astype(BF16)

    # offset-conv weights, output channels permuted to [dy*9, dx*9, m*9]
    perm = np.concatenate([np.arange(0, 17, 2), np.arange(1, 18, 2),
                           np.arange(18, 27)])
    wofp = w_off[perm]            # [27, CI, 3, 3]
    boffp = b_off[perm]
    woff_h = np.ascontiguousarray(
        wofp.reshape(27, 2, 128, 3, 3).transpose(2, 1, 3, 4, 0)
        .reshape(128, 2, 9, 27)).astype(BF16)

    # wdcn lhsT chunks: [p, ch=(k*2+cf), o2, oc] = w_dcn[o2*128+oc, cf*128+p, k]
    wd = w_dcn.reshape(CO, CI, 9)
    wdcn_h = np.ascontiguousarray(
        wd.reshape(2, 128, 2, 128, 9).transpose(3, 4, 2, 0, 1)
        .reshape(128, 9, 2, 2, 128).transpose(0, 1, 2, 3, 4)
        .reshape(128, 18, 2, 128)).astype(BF16)

    ident_h = np.eye(128, dtype=BF16)
    gb_h = np.zeros((128, 2, 3), np.float32)
    for o2 in range(2):
        gb_h[:, o2, 0] = gamma[o2 * 128:(o2 + 1) * 128]
        gb_h[:, o2, 1] = beta[o2 * 128:(o2 + 1) * 128]
        gb_h[:, o2, 2] = b_dcn[o2 * 128:(o2 + 1) * 128]

    in_maps = []
    for c in range(NCORES):
        b, rb = c // 4, c % 4
        slab_h = np.ascontiguousarray(
            xs[b].reshape(2, 128, H + 2, W + 2)
            .transpose(1, 0, 2, 3)[:, :, rb * RB:rb * RB + RB + 2, :])
        pypx_h = np.zeros((96, 24, 27), np.float32)
        pp = np.arange(96, dtype=np.float32)
        tt = np.arange(24, dtype=np.float32)
        pypx_h[:, :, 0:9] = (rb * RB - 1.0 + tt[None, :, None]
                             + KY9[None, None, :] + boffp[None, None, 0:9])
        pypx_h[:, :, 9:18] = (pp[:, None, None] - 1.0
                              + KX9[None, None, :] + boffp[None, None, 9:18])
        pypx_h[:, :, 18:27] = boffp[None, None, 18:27]
        in_maps.append({
            "tab": tabs[b], "slab": slab_h, "woff": woff_h,
            "pypx": pypx_h, "wdcn": wdcn_h, "ident": ident_h, "gb": gb_h,
        })
    return in_maps


def kernel(x, w_off, b_off, w_dcn, b_dcn, gamma, beta, _trace=False):
    from concourse.bass_utils import run_bass_kernel_spmd

    if "nc" not in _CACHE:
        _CACHE["nc"] = _build_program()
    nc = _CACHE["nc"]
    in_maps = _prep_inputs(x, w_off, b_off, w_dcn, b_dcn, gamma, beta)
    results = None
    try:
        try:
            res = run_bass_kernel_spmd(nc, in_maps,
                                       core_ids=list(range(NCORES)),
                                       trace=_trace)
        except ModuleNotFoundError:
            res = run_bass_kernel_spmd(nc, in_maps,
                                       core_ids=list(range(NCORES)),
                                       trace=False)
        _CACHE["last"] = res
        results = res.results
    except Exception:
        # hardware path unavailable: fall back to the multi-core simulator
        from concourse import bass_interp
        sim = bass_interp.MultiCoreSim(nc, NCORES)
        for c in range(NCORES):
            for name, val in in_maps[c].items():
                sim.cores[c].tensor(name)[:] = val
        sim.simulate()
        results = [{"out": np.asarray(sim.cores[c].tensor("out"))}
                   for c in range(NCORES)]
    out = np.empty((B, CO, H, W), np.float32)
    for c in range(NCORES):
        b, rb = c // 4, c % 4
        o = results[c]["out"]  # [2, 128, NPOS]
        out[b, :, rb * RB:(rb + 1) * RB, :] = o.reshape(CO, RB, W)
    return out

